# revision 1
# baseline (speedup 1.0000x reference)
"""Trainium2 Bass kernel for nn_ClusteringLayer (student-t soft assignment).

Math: q[b,k] = (1 + ||x_b - c_k||^2)^-1, out = q / q.sum(axis=1, keepdims=True)

Strategy (data-parallel over batch, 8 cores, 2048 rows each):
  1 + ||x-c||^2 = 1 + ||x||^2 + ||c||^2 - 2 x.c  is computed entirely on the
  PE array as a single accumulated matmul over an augmented contraction dim:
     xa[f,b]   = x^T            (f in 0..255)      ca[f,k]   = -2 c^T
     xa[256,b] = 1                                 ca[256,k] = 1 + ||c_k||^2
     xa[257,b] = ||x_b||^2                         ca[257,k] = 1
  so  psum[k,b] = sum_f ca[f,k] xa[f,b] = 1 + d2[k,b].
  The matmul runs in [clusters, batch] layout (moving free dim = batch = 512
  >= 256 lets float32r matmuls run at full PE rate).  Per 512-batch chunk:
     q = Exp(-Ln(psum))                           (ACT, [k, b])
     qT tiles via PE transpose                    (PE, [b, k] in PSUM)
     s = row-sums of qT, inv = 1/s                (DVE)
     o = qT * inv (per-partition scale)           (ACT)
  so the output is written in natural [batch, clusters] layout.
"""

import numpy as np

B = 16384
F = 256
K = 128
N_CORES = 8
BP = B // N_CORES  # 2048 rows per core
KAUG = F + 3  # 259 = 128 + 128 + 3 (ones, ||x||^2 hi, lo)
CHUNK = 512
N_CHUNKS = BP // CHUNK
TPC = CHUNK // 128  # transpose tiles per chunk


def _apply_tile_drain_patch():
    """This walrus build rejects >1 sync-wait command per instruction, but
    Tile's tail drain carries one wait per live semaphore.  Split them into
    individual sync.wait_ge instructions instead."""
    import concourse.tile as tile
    from concourse import mybir
    from concourse.vector_clock import ScopedClock

    def _drain_and_barrier_split(self, tick_clock, wait_clock):
        carrier = mybir.InstNoOp(
            name="detached-wait-carrier", ins=[], outs=[], engine=mybir.EngineType.SP
        )
        wait_clock.add_sem_waits(carrier, ScopedClock({None: tick_clock.global_clock}))
        waits = (
            list(carrier.sync_info.on_wait) if carrier.sync_info is not None else []
        )
        by_name = {}
        if self.sems is not None:
            for h in self.sems.allocated().values():
                by_name[getattr(h, "name", None)] = h
        for w in waits:
            h = by_name.get(w.ant_name)
            assert h is not None, (w.ant_name, list(by_name))
            self.nc.sync.wait_ge(h, w.wait_value)
        self.nc.sync.drain()
        self.nc.all_engine_barrier()
        assert self.sems is not None
        popped = self.nc._tile_sem_poison_stack.pop()
        assert popped is self._sem_poison
        self.nc.clear_and_free_semaphores(list(self.sems.allocated().values()))
        self.nc.all_engine_barrier()

    tile.TileContext._drain_and_barrier = _drain_and_barrier_split


def _split_multi_waits(nc):
    """This walrus build rejects instructions carrying more than one sync-wait
    command.  Hoist all but one wait of each instruction onto NoOp carriers
    inserted just before it on the same engine (the engine queue is in-order,
    so waiting on the NoOps first is equivalent)."""
    from concourse import mybir

    n_split = 0
    for func in nc.m.functions:
        for block in func.blocks:
            new_insts = []
            for inst in block.instructions:
                si = getattr(inst, "sync_info", None)
                waits = list(si.on_wait) if si is not None else []
                if len(waits) > 1:
                    for j, w in enumerate(waits[:-1]):
                        nop = mybir.InstNoOp(
                            name=f"{inst.name}-wsplit{j}",
                            ins=[],
                            outs=[],
                            engine=inst.engine,
                        )
                        nop.sync_info = mybir.SyncInfo(on_wait=[w], on_update=[])
                        new_insts.append(nop)
                    si.on_wait = [waits[-1]]
                    n_split += 1
                new_insts.append(inst)
            block.instructions = new_insts
    return n_split


def build_nc(split_waits=True):
    import concourse.bass as bass
    import concourse.tile as tile
    from concourse import bass_isa, mybir
    from concourse.masks import make_identity

    _apply_tile_drain_patch()

    f32 = mybir.dt.float32
    bf16 = mybir.dt.bfloat16

    nc = bass.Bass()
    xa = nc.dram_tensor("xa", [KAUG, BP], bf16, kind="ExternalInput")
    ca = nc.dram_tensor("ca", [KAUG, K], bf16, kind="ExternalInput")
    out = nc.dram_tensor("out", [BP, K], f32, kind="ExternalOutput")

    with tile.TileContext(nc) as tc:
        with (
            tc.tile_pool(name="consts", bufs=1) as consts,
            tc.tile_pool(name="xin", bufs=N_CHUNKS) as xin,
            tc.tile_pool(name="qp", bufs=2) as qp,
            tc.tile_pool(name="op", bufs=2) as op,
            tc.tile_pool(name="sp", bufs=2) as sp,
            tc.tile_pool(name="mm_ps", bufs=2, space="PSUM") as mm_ps,
            tc.tile_pool(name="qt_ps", bufs=4, space="PSUM") as qt_ps,
        ):
            ca0 = consts.tile([128, K], bf16)
            ca1 = consts.tile([128, K], bf16)
            ca2 = consts.tile([3, K], bf16)
            nc.sync.dma_start(out=ca0, in_=ca[0:128, :])
            nc.sync.dma_start(out=ca1, in_=ca[128:256, :])
            nc.sync.dma_start(out=ca2, in_=ca[256:259, :])
            ident = consts.tile([128, 128], f32)
            make_identity(nc, ident)

            for c in range(N_CHUNKS):
                sl = slice(c * CHUNK, (c + 1) * CHUNK)
                x0 = xin.tile([128, CHUNK], bf16, tag="x0")
                x1 = xin.tile([128, CHUNK], bf16, tag="x1")
                x2 = xin.tile([3, CHUNK], bf16, tag="x2")
                nc.sync.dma_start(out=x0, in_=xa[0:128, sl])
                nc.scalar.dma_start(out=x1, in_=xa[128:256, sl])
                nc.scalar.dma_start(out=x2, in_=xa[256:259, sl])

                ps = mm_ps.tile([128, CHUNK], f32, tag="ps")
                nc.tensor.matmul(ps, ca0, x0, start=True, stop=False)
                nc.tensor.matmul(ps, ca1, x1, start=False, stop=False)
                nc.tensor.matmul(ps, ca2, x2, start=False, stop=True)

                lq = qp.tile([128, CHUNK], f32, tag="lq")
                nc.scalar.activation(
                    out=lq, in_=ps, func=mybir.ActivationFunctionType.Ln
                )
                q = qp.tile([128, CHUNK], f32, tag="q")
                nc.scalar.activation(
                    out=q,
                    in_=lq,
                    func=mybir.ActivationFunctionType.Exp,
                    scale=-1.0,
                )

                o = op.tile([128, TPC, 128], f32, tag="o")
                for t in range(TPC):
                    qt = qt_ps.tile([128, 128], f32, tag="qt")
                    nc.tensor.transpose(
                        qt, q[:, t * 128 : (t + 1) * 128], ident
                    )
                    s = sp.tile([128, 1], f32, tag="s")
                    nc.vector.reduce_sum(out=s, in_=qt, axis=mybir.AxisListType.X)
                    inv = sp.tile([128, 1], f32, tag="inv")
                    nc.vector.reciprocal(out=inv, in_=s)
                    if t % 2 == 0:
                        nc.scalar.mul(o[:, t, :], qt, inv)
                    else:
                        nc.vector.tensor_scalar_mul(o[:, t, :], qt, inv)
                out_view = out[sl, :].rearrange("(t p) k -> p t k", p=128)
                nc.gpsimd.dma_start(out=out_view, in_=o)

    if split_waits:
        _split_multi_waits(nc)
    return nc


_NC_CACHE = None


def _get_nc():
    global _NC_CACHE
    if _NC_CACHE is None:
        _NC_CACHE = build_nc()
    return _NC_CACHE


def make_in_maps(inputs, clusters):
    X = np.ascontiguousarray(np.asarray(inputs, dtype=np.float32))
    C = np.ascontiguousarray(np.asarray(clusters, dtype=np.float32))
    assert X.shape == (B, F) and C.shape == (K, F), (X.shape, C.shape)
    import ml_dtypes

    bf16 = ml_dtypes.bfloat16
    xn = np.einsum("bf,bf->b", X, X, dtype=np.float32)
    cn = np.einsum("kf,kf->k", C, C, dtype=np.float32)
    xn_hi = xn.astype(bf16)
    xn_lo = (xn - xn_hi.astype(np.float32)).astype(bf16)
    ca = np.empty((KAUG, K), dtype=bf16)
    ca[:F] = (-2.0 * C).T.astype(bf16)
    ca[F] = (1.0 + cn).astype(bf16)
    ca[F + 1] = 1.0
    ca[F + 2] = 1.0
    in_maps = []
    for i in range(N_CORES):
        sl = slice(i * BP, (i + 1) * BP)
        xa = np.empty((KAUG, BP), dtype=bf16)
        xa[:F] = X[sl].T.astype(bf16)
        xa[F] = 1.0
        xa[F + 1] = xn_hi[sl]
        xa[F + 2] = xn_lo[sl]
        in_maps.append({"xa": xa, "ca": ca})
    return in_maps


def run(inputs, clusters, trace=False, tmpdir=None):
    """Run on 8 NeuronCores; returns (output, BassKernelResults)."""
    from concourse.bass_utils import run_bass_kernel_spmd

    in_maps = make_in_maps(inputs, clusters)
    nc = _get_nc()
    res = run_bass_kernel_spmd(
        nc, in_maps, list(range(N_CORES)), trace=trace, tmpdir=tmpdir
    )
    out = np.empty((B, K), dtype=np.float32)
    for i in range(N_CORES):
        out[i * BP : (i + 1) * BP] = res.results[i]["out"]
    return out, res


def kernel(inputs, clusters):
    out, _ = run(inputs, clusters, trace=False)
    return out

